# revision 18
# baseline (speedup 1.0000x reference)
"""Conv2DMod (StyleGAN2-style modulated conv) Trainium2 Bass kernel.

Problem: B=8, C_in=512, C_out=512, K=3x3, H=W=64, fp32, 'same' padding.

  wts[b,o,c,kh,kw] = weight[o,c,kh,kw] * (y[b,c]+1)
  d[b,o]           = rsqrt(sum_{c,kh,kw} wts^2 + 1e-8)
  out[b]           = conv2d(x[b], wts[b]*d[b,o])

Strategy (data-parallel over batch, one sample per NeuronCore, 8 cores):
  Since conv is linear in the weight and the modulation scale s_c=(y+1)
  depends only on the input channel while demod d_o only on the output
  channel:
      out = d_o * conv(s_c * x, weight)
  so each core:
    1. scales its x by s_c (per input channel),
    2. runs the 3x3 conv as 9 shift-matmuls against the *base* weight
       (host pre-transposed to [k, c, o] for natural lhsT tile loads),
       fp32r matmuls accumulating over k in PSUM, over c-tiles in SBUF,
    3. computes d_o on the PE (sum_c s2[c]*w2[c,o] via M=1 matmuls of
       squared weight tiles) and applies it on the PSUM->SBUF eviction.

kernel(x, y, weight) takes the FULL unsharded inputs and returns the
full (8, 512, 64, 64) fp32 output.
"""

import numpy as np

import concourse.bass as bass
import concourse.tile as tile
from concourse import bacc, mybir
from concourse.bass_utils import run_bass_kernel_spmd

# Problem constants (hardcoded per spec).
B = 8
C = 512          # input channels
O = 512          # output channels
H = W = 64
KK = 9           # 3x3 taps
PR = PW = 66     # padded image rows/cols
CT = 4           # c tiles of 128
OT = 4           # o tiles of 128
NCH = 8          # hw chunks: 8 rows x 64 cols = 512 free elems
ROWS = 8
EPS = 1e-8

F32 = mybir.dt.float32
F32R = mybir.dt.float32r
AF = mybir.ActivationFunctionType


def build_nc(reps=1):
    nc = bacc.Bacc(None, target_bir_lowering=False)

    x_d = nc.dram_tensor("x", [C, H, W], F32, kind="ExternalInput")
    y_d = nc.dram_tensor("y", [1, C], F32, kind="ExternalInput")
    wt_d = nc.dram_tensor("wt", [KK, C, O], F32, kind="ExternalInput")
    out_d = nc.dram_tensor("out", [O, H, W], F32, kind="ExternalOutput")

    with tile.TileContext(nc) as tc:
      for _rep in range(reps):
        with (
            tc.tile_pool(name="xpad", bufs=2) as xpad_pool,
            tc.tile_pool(name="xstg", bufs=2) as xstg_pool,
            tc.tile_pool(name="wl", bufs=4) as wl_pool,
            tc.tile_pool(name="wt", bufs=18) as wt_pool,
            tc.tile_pool(name="wt2", bufs=3) as wt2_pool,
            tc.tile_pool(name="acc", bufs=1) as acc_pool,
            tc.tile_pool(name="osb", bufs=4) as osb_pool,
            tc.tile_pool(name="small", bufs=1) as small_pool,
            tc.tile_pool(name="cpsum", bufs=6, space=bass.MemorySpace.PSUM) as cpsum_pool,
            tc.tile_pool(name="vpsum", bufs=1, space=bass.MemorySpace.PSUM) as vpsum_pool,
        ):
            # ---- y -> s = y+1 (row layout), transpose to per-partition cols
            y_sb = small_pool.tile([1, C], F32, tag="y")
            nc.sync.dma_start(y_sb[:], y_d[:])

            ones = small_pool.tile([1, 1], F32, tag="ones")
            nc.vector.memset(ones[:], 1.0)
            eps_1 = small_pool.tile([1, 1], F32, tag="eps1")
            nc.vector.memset(eps_1[:], EPS)
            zero_col = small_pool.tile([128, 1], F32, tag="zerocol")
            nc.vector.memset(zero_col[:], 0.0)

            s_row = small_pool.tile([1, C], F32, tag="srow")
            nc.scalar.activation(s_row[:], y_sb[:], AF.Identity,
                                 bias=ones[0:1, 0:1])

            # K=1 matmul transposes a row-vector slice into a psum column.
            s_col = small_pool.tile([128, CT], F32, tag="scol")
            for t in range(CT):
                ps = vpsum_pool.tile([128, 1], F32)
                nc.tensor.matmul(
                    ps[:], s_row[0:1, t * 128:(t + 1) * 128], ones[0:1, 0:1],
                    start=True, stop=True,
                )
                nc.scalar.copy(s_col[:, t:t + 1], ps[:])
            # fp32r matmul operands must be engine-produced with F32R
            # output dtype (HW rounds fp32 -> e11m8-in-high-20-bits).
            s2_col = small_pool.tile([128, CT], F32R, tag="s2col")
            nc.vector.tensor_mul(s2_col[:], s_col[:], s_col[:])

            d_acc = small_pool.tile([1, O], F32, tag="dacc")
            d_sq = small_pool.tile([1, O], F32, tag="dsq")
            d_row = small_pool.tile([1, O], F32, tag="drow")
            d_col = small_pool.tile([128, OT], F32, tag="dcol")

            # Persistent fp32 output accumulators: 32 tiles [128, 8, 64].
            acc_t = [
                [acc_pool.tile([128, ROWS, W], F32, name=f"acc_{ch}_{ot}",
                               tag=f"acc_{ch}_{ot}")
                 for ot in range(OT)]
                for ch in range(NCH)
            ]

            for ct in range(CT):
                c0 = ct * 128
                # -- weight tiles for this c-pass: wt[k][c0:c0+128, :]
                # DMA raw fp32, then engine-round to F32R for the PE.
                wts = []
                wls = []
                for k in range(KK):
                    wl = wl_pool.tile([128, O], F32, name="wl")
                    nc.sync.dma_start(wl[:], wt_d[k, c0:c0 + 128, :])
                    w_t = wt_pool.tile([128, O], F32R, name="w_t")
                    nc.vector.tensor_copy(w_t[:], wl[:])
                    wts.append(w_t)
                    wls.append(wl)

                # -- padded, s-scaled input image for this c-pass
                xp = xpad_pool.tile([128, PR, PW], F32R)
                nc.gpsimd.memset(xp[:].bitcast(F32), 0.0)
                # Stage raw fp32 rows, then scale by s_c while rounding to
                # F32R into the padded interior (separate src/dst keeps the
                # verifier's fp32r-producer rule happy).
                QR = 16  # rows per staging chunk
                for r0 in range(0, H, QR):
                    xs = xstg_pool.tile([128, QR, W], F32, name="xs")
                    nc.sync.dma_start(xs[:], x_d[c0:c0 + 128, r0:r0 + QR, :])
                    nc.vector.tensor_scalar_mul(
                        xp[:, 1 + r0:1 + r0 + QR, 1:PW - 1], xs[:],
                        s_col[:, ct:ct + 1])

                # -- demod partial: d_acc[o] += sum_c s2[c] * wt[k][c,o]^2
                ps_d = vpsum_pool.tile([1, O], F32)
                for k in range(KK):
                    w2 = wt2_pool.tile([128, O], F32R, name="w2")
                    nc.scalar.activation(w2[:], wls[k][:], AF.Square,
                                         bias=zero_col[:, 0:1])
                    nc.tensor.matmul(
                        ps_d[:],
                        s2_col[:, ct:ct + 1],
                        w2[:],
                        start=(k == 0), stop=(k == KK - 1),
                    )
                if ct == 0:
                    nc.scalar.copy(d_acc[:], ps_d[:])
                else:
                    nc.vector.tensor_add(d_acc[:], d_acc[:], ps_d[:])

                if ct == CT - 1:
                    # d = 1/sqrt(d_acc + eps), then transpose to columns.
                    nc.scalar.activation(d_sq[:], d_acc[:], AF.Sqrt,
                                         bias=eps_1[0:1, 0:1])
                    nc.vector.reciprocal(d_row[:], d_sq[:])
                    for t in range(OT):
                        ps = vpsum_pool.tile([128, 1], F32)
                        nc.tensor.matmul(
                            ps[:], d_row[0:1, t * 128:(t + 1) * 128],
                            ones[0:1, 0:1], start=True, stop=True,
                        )
                        nc.scalar.copy(d_col[:, t:t + 1], ps[:])

                # -- conv: 9 shift-matmuls accumulating over k in PSUM
                for ot in range(OT):
                    o0 = ot * 128
                    for q in range(2):
                        pss = [cpsum_pool.tile([128, ROWS, W], F32, name="convps")
                               for _ in range(4)]
                        for k in range(KK):
                            kh, kw = divmod(k, 3)
                            lhsT = wts[k][:, o0:o0 + 128]
                            for j in range(4):
                                ch = q * 4 + j
                                h0 = ch * ROWS
                                rhs = xp[:, h0 + kh:h0 + kh + ROWS,
                                         kw:kw + W]
                                nc.tensor.matmul(
                                    pss[j][:], lhsT, rhs,
                                    start=(k == 0), stop=(k == KK - 1),
                                )
                        for j in range(4):
                            ch = q * 4 + j
                            a = acc_t[ch][ot]
                            if ct == 0:
                                nc.scalar.copy(a[:], pss[j][:])
                            else:
                                nc.vector.tensor_add(a[:], a[:], pss[j][:])
                            if ct == CT - 1:
                                osb = osb_pool.tile([128, ROWS, W], F32)
                                nc.scalar.mul(osb[:], a[:],
                                              mul=d_col[:, ot:ot + 1])
                                nc.sync.dma_start(
                                    out_d[o0:o0 + 128,
                                          ch * ROWS:(ch + 1) * ROWS, :],
                                    osb[:],
                                )

    nc.compile()
    return nc


_CACHE = {}


def _get_nc():
    if "nc" not in _CACHE:
        _CACHE["nc"] = build_nc()
    return _CACHE["nc"]


def kernel(x, y, weight):
    x = np.ascontiguousarray(np.asarray(x, dtype=np.float32))
    y = np.ascontiguousarray(np.asarray(y, dtype=np.float32))
    weight = np.asarray(weight, dtype=np.float32)

    # [O, C, 3, 3] -> [9, C, O] so lhsT tiles ([c, o] per tap) DMA naturally.
    wt = np.ascontiguousarray(weight.transpose(2, 3, 1, 0).reshape(KK, C, O))

    nc = _get_nc()
    in_maps = [
        {"x": x[b], "y": y[b:b + 1], "wt": wt}
        for b in range(B)
    ]
    res = run_bass_kernel_spmd(nc, in_maps, core_ids=list(range(B)))
    kernel.last_results = res
    return np.stack([r["out"] for r in res.results], axis=0)


kernel.last_results = None
